# revision 54
# baseline (speedup 1.0000x reference)
"""Trainium2 Bass kernel for nn_Attention_3556232921308.

GQA attention layer: RMSNorm -> {Q+gate, K, V} proj -> softmax attention
(no mask, no rope) -> sigmoid output gate -> O proj.
B=2, S=2048, HID=2048, NH=16, NKV=4, HD=128.

Sharding (8 cores): DP over batch (2 groups of 4 cores) x TP over KV heads
(4 ranks per group; each rank owns 1 KV head = 4 Q/gate heads). The output
projection contracts over all heads, so gated attention outputs (bf16) are
exchanged with per-head AllGathers; each rank then computes the O-projection
for all tokens but only its quarter of the HID output columns (the Wo slice
is host-provided per rank, keeping the SPMD graph rank-independent).

Host-side prep: transposes (activations/weights enter the PE contracted
over the partition dim), folding the RMSNorm (1+w) scale into the
projection weights, pre-tiling weights into [P, KH*P] blocks so each loads
with one linear DMA, and casting hidden states + weights to bf16 (all
matmuls run bf16 except the exp(p)@v/sum path which stays float32r;
measured end-to-end rel err ~6e-3 vs the 2e-2 gate).

Compute layout notes:
 - hidden states live as hsT [HID, S]; mean-square is a ones-matvec on PE,
   and 1/rms is applied to the *outputs* of the raw projections (scaling by
   rstd commutes past the HID contraction), broadcast along partitions.
 - q/k are produced directly in [HD, S] (per head) layout, so scores^T
   [Sk, Sq] needs no transposes; softmax denominators are ones-matvecs.
 - v is produced as vT [HD, S] at full rate then PE-transposed per 128-tile.
 - exp(scores) runs on ACT straight out of PSUM with the 1/sqrt(HD) scale
   folded in; no max-subtraction (|scores| < 8 for unit-RMS inputs and
   0.02-scaled weights; fp32 exp is safe). The sigmoid gate is computed as
   1/(1+exp(-g)) so ACT never swaps activation tables in the hot loop.
"""
import math
from contextlib import ExitStack

import numpy as np

B, S_FULL, HID = 2, 2048, 2048
NH, NKV, HD = 16, 4, 128
G = NH // NKV  # 4 q heads per kv head = heads per rank
EPS = 1e-6
N_CORES = 8
P = 128
KH = HID // P  # 16 contraction tiles
HQ = HID // 4  # per-rank output column quarter (512)


def build(S=S_FULL):
    import concourse.bass as bass  # noqa: F401
    import concourse.tile as tile
    from concourse import bacc, mybir

    F32R = mybir.dt.float32r
    F32 = mybir.dt.float32
    BF16 = mybir.dt.bfloat16
    AF = mybir.ActivationFunctionType

    SQCH = S // 4  # attention sq chunk
    NW = min(512, S)  # projection free-dim chunk (psum bank = 512 fp32)
    NCH = S // NW
    MSW = NW  # mean-square matvec chunk
    MSCH = NCH
    NSK = S // P  # score key tiles
    HPR = G * HD  # feats per rank for q/gate (512)
    SCALE = 1.0 / math.sqrt(HD)
    RG = [[0, 1, 2, 3], [4, 5, 6, 7]]

    nc = bacc.Bacc("TRN2", target_bir_lowering=False, debug=False, num_devices=N_CORES)

    hst = nc.declare_dram_parameter("hst", [HID, S], BF16, isOutput=False)
    # weights ship pre-tiled as [P, KH*P] blocks (one linear DMA each)
    wqt = nc.declare_dram_parameter("wqt", [2 * G, P, KH * P], BF16, isOutput=False)
    wkt = nc.declare_dram_parameter("wkt", [P, KH * P], BF16, isOutput=False)
    wvt = nc.declare_dram_parameter("wvt", [P, KH * P], BF16, isOutput=False)
    wot = nc.declare_dram_parameter("wot", [NH * HD, HQ], BF16, isOutput=False)
    identp = nc.declare_dram_parameter("identp", [P, P], BF16, isOutput=False)
    out = nc.declare_dram_parameter("out", [HQ, S], F32, isOutput=True)

    with tile.TileContext(nc) as tc, ExitStack() as ctx:
        dram = ctx.enter_context(tc.tile_pool(name="dram", bufs=1, space="DRAM"))
        # every head's AllGather is split into two column-halves: the first
        # half launches mid-head and its ~20us exchange hides under the rest
        # of the head's attention, so neither the next head nor the final
        # O-projection ever waits on a full 2MB end-of-head exchange
        ag_in = [
            [
                dram.tile([P, S // 2], BF16, name=f"ag_in{h}_{i}", uniquify=False)
                for i in range(2)
            ]
            for h in range(G)
        ]
        ag_out = [
            [
                dram.tile(
                    [4 * P, S // 2], BF16, name=f"ag_out{h}_{i}", uniquify=False
                )
                for i in range(2)
            ]
            for h in range(G)
        ]

        warm_in = dram.tile([P, S // 2], BF16)
        warm_out = dram.tile([4 * P, S // 2], BF16)

        consts = ctx.enter_context(tc.tile_pool(name="consts", bufs=1))
        ones_sb = consts.tile([P, 1], BF16)
        nc.vector.memset(ones_sb[:], 1.0)
        ident_sb = consts.tile([P, P], BF16)
        nc.gpsimd.dma_start(out=ident_sb[:], in_=identp[:])
        rstd_bc = consts.tile([P, S], F32)
        eps_t = consts.tile([1, 1], F32)
        nc.vector.memset(eps_t[:], EPS)

        with ExitStack() as ph123:
            kv_pool = ph123.enter_context(tc.tile_pool(name="kv", bufs=1))
            kt_sb = kv_pool.tile([P, S], BF16)
            vnat = [kv_pool.tile([P, P], BF16, name=f"vnat{i}", uniquify=False)
                    for i in range(NSK)]
            # q/gate stay SBUF-resident (evac writes land here directly;
            # no DRAM round trip, no attention-phase reload DMAs)
            qt_sb = [kv_pool.tile([P, S], BF16, name=f"qt{m}", uniquify=False)
                     for m in range(G)]
            gate_sb = [kv_pool.tile([P, S], BF16, name=f"gt{m}", uniquify=False)
                       for m in range(G)]

            # ---- phases 1+2: norm stats + projections (hsT resident) ----
            with ExitStack() as ph:
                ht_pool = ph.enter_context(tc.tile_pool(name="ht", bufs=1))
                ht = [ht_pool.tile([P, S], BF16, name=f"ht{k}", uniquify=False)
                      for k in range(KH)]
                vt_sb = ht_pool.tile([P, S], BF16)

                # norm stats + projections. The mean-square matvecs are
                # software-pipelined one k behind their squares (half on
                # ACT, half on DVE), with the m=0 q-projection psum chains
                # interleaved so the PE never idles waiting on squares.
                with tc.tile_pool(name="sqp", bufs=4) as sqp, tc.tile_pool(
                    name="wq", bufs=2
                ) as wqp:

                    def load_wq(m, split=False):
                        wq_m = wqp.tile([P, KH, P], BF16, name="wq_m", tag="wq_m")
                        dma_eng = nc.sync if m % 2 == 0 else nc.scalar
                        src = wqt[m].rearrange("p (k j) -> p k j", k=KH)
                        if split:
                            # small head piece first so the k=0 chain step
                            # can fire while the bulk is still streaming
                            nc.scalar.dma_start(
                                out=wq_m[:, 0:2, :], in_=src[:, 0:2, :]
                            )
                            nc.scalar.dma_start(
                                out=wq_m[:, 2:, :], in_=src[:, 2:, :]
                            )
                        else:
                            dma_eng.dma_start(out=wq_m[:, :, :], in_=src)
                        return wq_m

                    def qg_chain_mm(ps, wq_m, k, n):
                        nc.tensor.matmul(
                            ps[:],
                            wq_m[:, k, :],
                            ht[k][:, n * NW:(n + 1) * NW],
                            start=(k == 0),
                            stop=(k == KH - 1),
                        )

                    def evac(ps, m, n):
                        nsl = slice(n * NW, (n + 1) * NW)
                        if m < G:
                            nc.vector.tensor_mul(
                                qt_sb[m][:, nsl], ps[:], rstd_bc[:, nsl]
                            )
                        else:
                            # gate heads: apply the sigmoid here on ACT (one
                            # table load for all of phase 2) so the attention
                            # hot loop never touches the gate nonlinearity
                            gtmp = sqp.tile([P, NW], F32, name="gtmp")
                            nc.vector.tensor_mul(
                                gtmp[:], ps[:], rstd_bc[:, nsl]
                            )
                            nc.scalar.activation(
                                gate_sb[m - G][:, nsl], gtmp[:], AF.Sigmoid
                            )

                    qgps0_cm = tc.tile_pool(name="qgps0", bufs=1, space="PSUM")
                    qgps0 = qgps0_cm.__enter__()
                    with tc.tile_pool(name="msp", bufs=1, space="PSUM") as msp:
                        ms_ps = [msp.tile([1, MSW], F32, name=f"ms{n}",
                                          uniquify=False) for n in range(MSCH)]
                        # ht[0] leads on the sync queue (the k=0 chain+square
                        # need it first) while wq_0 streams on scalar with a
                        # small head piece; remaining tiles round-robin
                        ht_engs = [nc.sync, nc.gpsimd, nc.scalar]
                        ht_engs[0].dma_start(
                            out=ht[0][:], in_=hst[0:P, :]
                        )
                        wq_0 = load_wq(0, split=True)
                        for k in range(1, KH):
                            ht_engs[k % 3].dma_start(
                                out=ht[k][:], in_=hst[k * P:(k + 1) * P, :]
                            )
                        # tiny warmup collective (after the ht loads so it
                        # doesn't head-of-line-block them): absorbs NRT
                        # collective-channel init + cross-core launch skew;
                        # same byte-size as the real half-head AllGathers
                        nc.gpsimd.dma_start(
                            out=warm_in[:], in_=hst[0:P, 0:S // 2]
                        )
                        nc.gpsimd.collective_compute(
                            "AllGather",
                            mybir.AluOpType.bypass,
                            replica_groups=RG,
                            ins=[warm_in[:].opt()],
                            outs=[warm_out[:].opt()],
                        )
                        # k-outer chains: all NCH column chunks accumulate in
                        # parallel psum banks so each weight k-slice is loaded
                        # into the PE array once per m (4 matmuls per swap)
                        ps_m0 = [qgps0.tile([P, NW], F32, name=f"psq{n}")
                                 for n in range(NCH)]
                        sq_prev = None
                        for k in range(KH):
                            sq_k = []
                            for n in range(MSCH):
                                sqk = sqp.tile([P, MSW], BF16)
                                src = ht[k][:, n * MSW:(n + 1) * MSW]
                                nc.vector.tensor_mul(sqk[:], src, src)
                                sq_k.append(sqk)
                            for n in range(len(ps_m0)):
                                qg_chain_mm(ps_m0[n], wq_0, k, n)
                            if sq_prev is not None:
                                for n in range(MSCH):
                                    nc.tensor.matmul(
                                        ms_ps[n][:],
                                        ones_sb[:],
                                        sq_prev[n][:],
                                        start=(k == 1),
                                        stop=(k == KH - 1 + 1),
                                    )
                            sq_prev = sq_k
                        for n in range(MSCH):
                            nc.tensor.matmul(
                                ms_ps[n][:], ones_sb[:], sq_prev[n][:],
                                start=False, stop=True,
                            )
                        # rstd = exp(-0.5*ln(ms)) on ACT (Rsqrt is blocked in
                        # bass). All Ln chunks run before all Exp chunks so
                        # the ACT table loads twice, not per chunk; the
                        # broadcast is per-chunk so evacs start early.
                        srow = sqp.tile([1, S], F32, bufs=1)
                        lrow = sqp.tile([1, S], F32, bufs=1)
                        for n in range(MSCH):
                            msl = slice(n * MSW, (n + 1) * MSW)
                            nc.scalar.activation(
                                lrow[:, msl],
                                ms_ps[n][:],
                                AF.Ln,
                                bias=eps_t[:],
                                scale=1.0 / HID,
                            )
                        for n in range(MSCH):
                            msl = slice(n * MSW, (n + 1) * MSW)
                            nc.scalar.activation(
                                srow[:, msl], lrow[:, msl], AF.Exp, scale=-0.5
                            )
                            nc.gpsimd.partition_broadcast(
                                rstd_bc[:, msl], srow[:, msl]
                            )
                        for n in range(len(ps_m0)):
                            evac(ps_m0[n], 0, n)
                    qgps0_cm.__exit__(None, None, None)
                    qgps_cm = tc.tile_pool(name="qgps", bufs=2, space="PSUM")
                    qgps = qgps_cm.__enter__()

                    for m in range(1, 2 * G):
                        wq_m = load_wq(m)
                        pss = [qgps.tile([P, NW], F32, name=f"psq_r{n}")
                               for n in range(NCH)]
                        for k in range(KH):
                            for n in range(NCH):
                                qg_chain_mm(pss[n], wq_m, k, n)
                        for n in range(NCH):
                            evac(pss[n], m, n)

                    # k and v (vT), rstd-scaled at evacuation; the weight
                    # tiles share the wq_m rotation slots
                    wk_sb = wqp.tile([P, KH, P], BF16, name="wk_sb", tag="wq_m")
                    wv_sb = wqp.tile([P, KH, P], BF16, name="wv_sb", tag="wq_m")
                    nc.sync.dma_start(
                        out=wk_sb[:, :, :],
                        in_=wkt[:].rearrange("p (k j) -> p k j", k=KH),
                    )
                    nc.scalar.dma_start(
                        out=wv_sb[:, :, :],
                        in_=wvt[:].rearrange("p (k j) -> p k j", k=KH),
                    )
                    qgps_cm.__exit__(None, None, None)
                    with tc.tile_pool(name="kvps", bufs=2, space="PSUM") as kvps:
                     for dst_sb, w_sb in ((kt_sb, wk_sb), (vt_sb, wv_sb)):
                        pss = [kvps.tile([P, NW], F32, name=f"ps_kv{n}")
                               for n in range(NCH)]
                        for k in range(KH):
                            for n in range(NCH):
                                nc.tensor.matmul(
                                    pss[n][:],
                                    w_sb[:, k, :],
                                    ht[k][:, n * NW:(n + 1) * NW],
                                    start=(k == 0),
                                    stop=(k == KH - 1),
                                )
                        for n in range(NCH):
                            nc.vector.tensor_mul(
                                dst_sb[:, n * NW:(n + 1) * NW],
                                pss[n][:],
                                rstd_bc[:, n * NW:(n + 1) * NW],
                            )

                # v natural layout via PE transpose of vT tiles
                with tc.tile_pool(name="tpps", bufs=2, space="PSUM") as tpps:
                    for sk in range(NSK):
                        pst = tpps.tile([P, P], BF16)
                        nc.tensor.transpose(
                            pst[:], vt_sb[:, sk * P:(sk + 1) * P], ident_sb[:]
                        )
                        nc.vector.tensor_copy(vnat[sk][:], pst[:])

            # ---- phases 3+4 pools (allocated in the freed hsT zone) ----
            with ExitStack() as ph34:
                wo_bfp = ph34.enter_context(tc.tile_pool(name="wo_bf", bufs=1))
                wo_bf = [wo_bfp.tile([P, HQ], BF16, name=f"wo{kf}", uniquify=False)
                         for kf in range(KH)]
                of_pool = ph34.enter_context(tc.tile_pool(name="of", bufs=1))
                of = [of_pool.tile([P, S], BF16, name=f"of{i}", uniquify=False)
                      for i in range(KH)]
                for kf in range(KH):
                    dma_eng = nc.sync if kf % 2 == 0 else nc.scalar
                    dma_eng.dma_start(
                        out=wo_bf[kf][:], in_=wot[kf * P:(kf + 1) * P, :]
                    )

                # ---- phase 3: attention ----
                with tc.tile_pool(
                    name="pt", bufs=3
                ) as ptp, tc.tile_pool(name="og", bufs=2) as ogp, tc.tile_pool(
                    name="sps", bufs=2, space="PSUM"
                ) as sps, tc.tile_pool(
                    name="ops", bufs=2, space="PSUM"
                ) as ops, tc.tile_pool(name="sums", bufs=2, space="PSUM") as sums:
                    def emit_of_loads(hh):
                        # deferred until the exchange is surely complete, so
                        # these never head-of-line-block the gpsimd queue
                        for half in range(2):
                            hsl = slice(half * (S // 2), (half + 1) * (S // 2))
                            for r in range(4):
                                nc.gpsimd.dma_start(
                                    out=of[hh * 4 + r][:, hsl],
                                    in_=ag_out[hh][half][r * P:(r + 1) * P, :],
                                )

                    for h in range(G):
                        for sqc in range(4):
                            ssl = slice(sqc * SQCH, (sqc + 1) * SQCH)
                            if sqc == 2 and h > 0:
                                emit_of_loads(h - 1)
                            ps_o = ops.tile([P, SQCH], F32)
                            ps_sum = sums.tile([1, SQCH], F32)

                            # pairs of sk tiles share one 2-bank psum + one
                            # exp; p@v of pair skp-1 is emitted after the
                            # scores of pair skp so the PE never sits behind
                            # the ACT exp on the critical path. The pair is
                            # pre-added on DVE so the rowsum matvec streams
                            # once per pair instead of once per sk tile.
                            def emit_pv(skp, pt, pts):
                                for j in range(2):
                                    sk = 2 * skp + j
                                    nc.tensor.matmul(
                                        ps_o[:],
                                        vnat[sk][:],
                                        pt[:, j, :],
                                        start=(sk == 0),
                                        stop=(sk == NSK - 1),
                                    )
                                nc.tensor.matmul(
                                    ps_sum[:],
                                    ones_sb[:],
                                    pts[:],
                                    start=(skp == 0),
                                    stop=(skp == NSK // 2 - 1),
                                )

                            pend = None
                            for skp in range(NSK // 2):
                                ps_s = sps.tile([P, 2, SQCH], F32)
                                for j in range(2):
                                    sk = 2 * skp + j
                                    nc.tensor.matmul(
                                        ps_s[:, j, :],
                                        kt_sb[:, sk * P:(sk + 1) * P],
                                        qt_sb[h][:, ssl],
                                        start=True,
                                        stop=True,
                                    )
                                pt = ptp.tile([P, 2, SQCH], BF16)
                                nc.scalar.activation(
                                    pt[:], ps_s[:], AF.Exp, scale=SCALE
                                )
                                pts = ptp.tile([P, SQCH], BF16, name="pts")
                                nc.vector.tensor_add(
                                    pts[:], pt[:, 0, :], pt[:, 1, :]
                                )
                                if pend is not None:
                                    emit_pv(*pend)
                                pend = (skp, pt, pts)
                            emit_pv(*pend)

                            # normalization: gate already sigmoided in phase
                            # 2, so t1 = ps_o*sig frees the psum bank with no
                            # dependence on the rowsum path; the rowsum
                            # broadcast stays on-chip via partition_broadcast
                            # (no DRAM round trip, no sync-queue coupling)
                            rs = ogp.tile([1, SQCH], F32)
                            nc.vector.tensor_copy(rs[:], ps_sum[:])
                            rb = ogp.tile([P, SQCH], F32)
                            nc.gpsimd.partition_broadcast(rb[:], rs[:])
                            t1 = ogp.tile([P, SQCH], F32)
                            nc.vector.tensor_mul(
                                t1[:], ps_o[:], gate_sb[h][:, ssl]
                            )
                            rbr = ogp.tile([P, SQCH], F32)
                            nc.vector.reciprocal(rbr[:], rb[:])
                            og = ogp.tile([P, SQCH], BF16)
                            nc.gpsimd.tensor_mul(og[:], t1[:], rbr[:])
                            nc.sync.dma_start(
                                out=ag_in[h][sqc // 2][
                                    :, (sqc % 2) * SQCH:(sqc % 2 + 1) * SQCH
                                ],
                                in_=og[:],
                            )
                            if sqc % 2 == 1:
                                half = sqc // 2
                                nc.gpsimd.collective_compute(
                                    "AllGather",
                                    mybir.AluOpType.bypass,
                                    replica_groups=RG,
                                    ins=[ag_in[h][half][:].opt()],
                                    outs=[ag_out[h][half][:].opt()],
                                )
                    emit_of_loads(G - 1)

                # ---- phase 4: O projection (bf16), my HID column quarter ----
                # phase A contracts heads 0..2 (kf 0..11) into SBUF right
                # after head-3 attention, without waiting on the last
                # AllGather; phase B adds head 3 and writes out
                with tc.tile_pool(name="oacc", bufs=1) as oaccp, \
                        tc.tile_pool(name="oev", bufs=3) as oevp:
                    NM = HQ // P
                    OW = min(512, S)  # o-proj chunk (psum: NM banks/set)
                    OCH = S // OW
                    oacc = [
                        oaccp.tile([P, OW], F32, name=f"oacc{n}_{m}")
                        for n in range(OCH) for m in range(NM)
                    ]
                    # phase A runs on 2-matmul psum sets (4 banks total): it
                    # can grab the freed score psum banks and start while the
                    # last head's og tail still holds the ops/sums banks
                    with tc.tile_pool(name="outpsA", bufs=2, space="PSUM") \
                            as outpsA:
                        for n in range(OCH):
                            for mh in range(2):
                                pss = [
                                    outpsA.tile([P, OW], F32, name=f"opsA{mi}")
                                    for mi in range(2)
                                ]
                                for kf in range(12):
                                    for mi in range(2):
                                        m = mh * 2 + mi
                                        nc.tensor.matmul(
                                            pss[mi][:],
                                            wo_bf[kf][:, m * P:(m + 1) * P],
                                            of[kf][:, n * OW:(n + 1) * OW],
                                            start=(kf == 0),
                                            stop=(kf == 11),
                                        )
                                for mi in range(2):
                                    nc.vector.tensor_copy(
                                        oacc[n * NM + mh * 2 + mi][:],
                                        pss[mi][:],
                                    )
                    outps_cm = tc.tile_pool(name="outps", bufs=2, space="PSUM")
                    outps = outps_cm.__enter__()
                    for n in range(OCH):
                        pss = [outps.tile([P, OW], F32, name=f"ops{m}")
                               for m in range(NM)]
                        for kf in range(12, KH):
                            for m in range(NM):
                                nc.tensor.matmul(
                                    pss[m][:],
                                    wo_bf[kf][:, m * P:(m + 1) * P],
                                    of[kf][:, n * OW:(n + 1) * OW],
                                    start=(kf == 12),
                                    stop=(kf == KH - 1),
                                )
                        for m in range(NM):
                            oev = oevp.tile([P, OW], F32)
                            nc.vector.tensor_add(
                                oev[:], pss[m][:], oacc[n * NM + m][:]
                            )
                            dma_eng = nc.sync if (n + m) % 2 == 0 else nc.scalar
                            dma_eng.dma_start(
                                out=out[m * P:(m + 1) * P, n * OW:(n + 1) * OW],
                                in_=oev[:],
                            )
                    outps_cm.__exit__(None, None, None)

    nc.compile()
    return nc


def make_in_maps(hidden_states, Wq, Wk, Wv, Wo, norm_w, S=S_FULL):
    """Host-side sharding/layout prep. Core c -> (batch c//4, rank c%4)."""
    w1p = (1.0 + norm_w).astype(np.float32)
    WqT = np.ascontiguousarray((Wq * w1p[None, :]).T)  # [HID, 2*NH*HD]
    WkT = np.ascontiguousarray((Wk * w1p[None, :]).T)  # [HID, NKV*HD]
    WvT = np.ascontiguousarray((Wv * w1p[None, :]).T)
    WoT = np.ascontiguousarray(Wo.T)  # [NH*HD, HID]
    # permute feat blocks to match AG stacking: pos h*4+r holds head 4r+h
    perm = [4 * (p % 4) + p // 4 for p in range(NH)]
    WoTp = np.ascontiguousarray(
        WoT.reshape(NH, HD, HID)[perm].reshape(NH * HD, HID)
    )
    ident = np.eye(P, dtype=np.float32)

    def tile_w(wt):
        # [HID, C] -> per 128-col block m: [P, KH*P] with wq_m[p, k*P+j] =
        # wt[k*P+p, m*P+j]
        C = wt.shape[1]
        blocks = []
        for m in range(C // P):
            blk = wt[:, m * P:(m + 1) * P].reshape(KH, P, P)
            blocks.append(blk.transpose(1, 0, 2).reshape(P, KH * P))
        return np.ascontiguousarray(np.stack(blocks))

    import ml_dtypes

    bf = ml_dtypes.bfloat16
    in_maps = []
    for c in range(N_CORES):
        b, r = c // 4, c % 4
        qcols = np.r_[r * 512:(r + 1) * 512, NH * HD + r * 512:NH * HD + (r + 1) * 512]
        in_maps.append(
            {
                "hst": np.ascontiguousarray(hidden_states[b, :S].T.astype(bf)),
                "wqt": tile_w(WqT[:, qcols]).astype(bf),
                "wkt": tile_w(WkT[:, r * HD:(r + 1) * HD])[0].astype(bf),
                "wvt": tile_w(WvT[:, r * HD:(r + 1) * HD])[0].astype(bf),
                "wot": np.ascontiguousarray(
                    WoTp[:, r * HQ:(r + 1) * HQ].astype(bf)
                ),
                "identp": ident.astype(bf),
            }
        )
    return in_maps


def gather_out(results, S=S_FULL):
    out = np.empty((B, S, HID), np.float32)
    for c in range(N_CORES):
        b, r = c // 4, c % 4
        out[b, :, r * HQ:(r + 1) * HQ] = results[c]["out"].T
    return out


_NC_CACHE = {}


def kernel(**inputs) -> np.ndarray:
    from concourse.bass_utils import run_bass_kernel_spmd

    hidden_states = np.asarray(inputs["hidden_states"], dtype=np.float32)
    Wq = np.asarray(inputs["Wq"], dtype=np.float32)
    Wk = np.asarray(inputs["Wk"], dtype=np.float32)
    Wv = np.asarray(inputs["Wv"], dtype=np.float32)
    Wo = np.asarray(inputs["Wo"], dtype=np.float32)
    norm_w = np.asarray(inputs["norm_w"], dtype=np.float32)

    if "nc" not in _NC_CACHE:
        _NC_CACHE["nc"] = build()
    nc = _NC_CACHE["nc"]

    in_maps = make_in_maps(hidden_states, Wq, Wk, Wv, Wo, norm_w)
    res = run_bass_kernel_spmd(nc, in_maps, list(range(N_CORES)))
    return gather_out(res.results)



# revision 66
# speedup vs baseline: 1.1135x; 1.1135x over previous
"""Trainium2 Bass kernel for nn_Attention_3556232921308.

GQA attention layer: RMSNorm -> {Q+gate, K, V} proj -> softmax attention
(no mask, no rope) -> sigmoid output gate -> O proj.
B=2, S=2048, HID=2048, NH=16, NKV=4, HD=128.

Sharding (8 cores): DP over batch (2 groups of 4 cores) x TP over KV heads
(4 ranks per group; each rank owns 1 KV head = 4 Q/gate heads). The output
projection contracts over all heads, so gated attention outputs (bf16) are
exchanged with per-head AllGathers; each rank then computes the O-projection
for all tokens but only its quarter of the HID output columns (the Wo slice
is host-provided per rank, keeping the SPMD graph rank-independent).

Host-side prep: transposes (activations/weights enter the PE contracted
over the partition dim), folding the RMSNorm (1+w) scale into the
projection weights, pre-tiling weights into [P, KH*P] blocks so each loads
with one linear DMA, and casting hidden states + weights to bf16 (all
matmuls run bf16 except the exp(p)@v/sum path which stays float32r;
measured end-to-end rel err ~6e-3 vs the 2e-2 gate).

Compute layout notes:
 - hidden states live as hsT [HID, S]; mean-square is a ones-matvec on PE,
   and 1/rms is applied to the *outputs* of the raw projections (scaling by
   rstd commutes past the HID contraction), broadcast along partitions.
 - q/k are produced directly in [HD, S] (per head) layout, so scores^T
   [Sk, Sq] needs no transposes; softmax denominators are ones-matvecs.
 - v is produced as vT [HD, S] at full rate then PE-transposed per 128-tile.
 - exp(scores) runs on ACT straight out of PSUM with the 1/sqrt(HD) scale
   folded in; no max-subtraction (|scores| < 8 for unit-RMS inputs and
   0.02-scaled weights; fp32 exp is safe). The sigmoid gate is computed as
   1/(1+exp(-g)) so ACT never swaps activation tables in the hot loop.
"""
import math
from contextlib import ExitStack

import numpy as np

B, S_FULL, HID = 2, 2048, 2048
NH, NKV, HD = 16, 4, 128
G = NH // NKV  # 4 q heads per kv head = heads per rank
EPS = 1e-6
N_CORES = 8
P = 128
KH = HID // P  # 16 contraction tiles
HQ = HID // 4  # per-rank output column quarter (512)


def build(S=S_FULL):
    import concourse.bass as bass  # noqa: F401
    import concourse.tile as tile
    from concourse import bacc, mybir

    F32R = mybir.dt.float32r
    F32 = mybir.dt.float32
    BF16 = mybir.dt.bfloat16
    AF = mybir.ActivationFunctionType

    SQCH = S // 4  # attention sq chunk
    NW = min(512, S)  # projection free-dim chunk (psum bank = 512 fp32)
    NCH = S // NW
    MSW = NW  # mean-square matvec chunk
    MSCH = NCH
    NSK = S // P  # score key tiles
    HPR = G * HD  # feats per rank for q/gate (512)
    SCALE = 1.0 / math.sqrt(HD)
    RG = [[0, 1, 2, 3], [4, 5, 6, 7]]

    nc = bacc.Bacc("TRN2", target_bir_lowering=False, debug=False, num_devices=N_CORES)

    hst = nc.declare_dram_parameter("hst", [HID, S], BF16, isOutput=False)
    # weights ship pre-tiled as [P, KH*P] blocks (one linear DMA each)
    wqt = nc.declare_dram_parameter("wqt", [2 * G, P, KH * P], BF16, isOutput=False)
    wkt = nc.declare_dram_parameter("wkt", [P, KH * P], BF16, isOutput=False)
    wvt = nc.declare_dram_parameter("wvt", [P, KH * P], BF16, isOutput=False)
    wot = nc.declare_dram_parameter("wot", [NH * HD, HQ], BF16, isOutput=False)
    identp = nc.declare_dram_parameter("identp", [P, P], BF16, isOutput=False)
    out = nc.declare_dram_parameter("out", [HQ, S], F32, isOutput=True)

    with tile.TileContext(nc) as tc, ExitStack() as ctx:
        dram = ctx.enter_context(tc.tile_pool(name="dram", bufs=1, space="DRAM"))
        # every head's AllGather is split into two column-halves: the first
        # half launches mid-head and its ~20us exchange hides under the rest
        # of the head's attention, so neither the next head nor the final
        # O-projection ever waits on a full 2MB end-of-head exchange
        ag_in = [
            [
                dram.tile([P, S // 2], BF16, name=f"ag_in{h}_{i}", uniquify=False)
                for i in range(2)
            ]
            for h in range(G)
        ]
        ag_out = [
            [
                dram.tile(
                    [4 * P, S // 2], BF16, name=f"ag_out{h}_{i}", uniquify=False
                )
                for i in range(2)
            ]
            for h in range(G)
        ]

        warm_in = dram.tile([P, S // 2], BF16)
        warm_out = dram.tile([4 * P, S // 2], BF16)

        consts = ctx.enter_context(tc.tile_pool(name="consts", bufs=1))
        ones_sb = consts.tile([P, 1], BF16)
        nc.vector.memset(ones_sb[:], 1.0)
        # all-ones stationary: the softmax rowsum matvec writes its result
        # broadcast across all 128 psum partitions (same cost: matmul time
        # scales with moving columns only), so no copy/broadcast plumbing
        ones_mat = consts.tile([P, P], BF16)
        nc.vector.memset(ones_mat[:], 1.0)
        ident_sb = consts.tile([P, P], BF16)
        nc.gpsimd.dma_start(out=ident_sb[:], in_=identp[:])
        rstd_bc = consts.tile([P, S], F32)
        eps_t = consts.tile([1, 1], F32)
        nc.vector.memset(eps_t[:], EPS)

        # PE pre-warm: a burst of 1-column dummy matmuls while the first
        # weight/activation DMAs stream in, so the HAM clock gate reaches
        # 8/8 before the first real chain issues (saves the 1.2GHz ramp)
        with tc.tile_pool(name="pwm", bufs=1, space="PSUM") as pwmp:
            pwt = pwmp.tile([1, 1], F32)
            for _ in range(40):
                nc.tensor.matmul(
                    pwt[:], ones_sb[:, :1], ones_sb[:, :1],
                    start=True, stop=True,
                )

        with ExitStack() as ph123:
            kv_pool = ph123.enter_context(tc.tile_pool(name="kv", bufs=1))
            kt_sb = kv_pool.tile([P, S], BF16)
            vnat = [kv_pool.tile([P, P], BF16, name=f"vnat{i}", uniquify=False)
                    for i in range(NSK)]
            # q/gate stay SBUF-resident (evac writes land here directly;
            # no DRAM round trip, no attention-phase reload DMAs)
            qt_sb = [kv_pool.tile([P, S], BF16, name=f"qt{m}", uniquify=False)
                     for m in range(G)]
            gate_sb = [kv_pool.tile([P, S], BF16, name=f"gt{m}", uniquify=False)
                       for m in range(G)]

            # ---- phases 1+2: norm stats + projections (hsT resident) ----
            with ExitStack() as ph:
                ht_pool = ph.enter_context(tc.tile_pool(name="ht", bufs=1))
                ht = [ht_pool.tile([P, S], BF16, name=f"ht{k}", uniquify=False)
                      for k in range(KH)]
                vt_sb = ht_pool.tile([P, S], BF16)

                # norm stats + projections. The mean-square matvecs are
                # software-pipelined one k behind their squares (half on
                # ACT, half on DVE), with the m=0 q-projection psum chains
                # interleaved so the PE never idles waiting on squares.
                with tc.tile_pool(name="sqp", bufs=4) as sqp, tc.tile_pool(
                    name="wq", bufs=2
                ) as wqp, tc.tile_pool(name="kvw", bufs=1) as kvwp:

                    def load_wq(m, split=False):
                        wq_m = wqp.tile([P, KH, P], BF16, name="wq_m", tag="wq_m")
                        dma_eng = nc.sync if m % 2 == 0 else nc.scalar
                        src = wqt[m].rearrange("p (k j) -> p k j", k=KH)
                        if split:
                            # small head piece first so the k=0 chain step
                            # can fire while the bulk is still streaming
                            nc.scalar.dma_start(
                                out=wq_m[:, 0:2, :], in_=src[:, 0:2, :]
                            )
                            nc.scalar.dma_start(
                                out=wq_m[:, 2:, :], in_=src[:, 2:, :]
                            )
                        else:
                            dma_eng.dma_start(out=wq_m[:, :, :], in_=src)
                        return wq_m

                    def qg_chain_mm(ps, wq_m, k, n):
                        nc.tensor.matmul(
                            ps[:],
                            wq_m[:, k, :],
                            ht[k][:, n * NW:(n + 1) * NW],
                            start=(k == 0),
                            stop=(k == KH - 1),
                        )

                    def evac(ps, m, n):
                        nsl = slice(n * NW, (n + 1) * NW)
                        if m < G:
                            nc.vector.tensor_mul(
                                qt_sb[m][:, nsl], ps[:], rstd_bc[:, nsl]
                            )
                        else:
                            # gate heads: apply the sigmoid here on ACT (one
                            # table load for all of phase 2) so the attention
                            # hot loop never touches the gate nonlinearity
                            gtmp = sqp.tile([P, NW], F32, name="gtmp")
                            nc.vector.tensor_mul(
                                gtmp[:], ps[:], rstd_bc[:, nsl]
                            )
                            nc.scalar.activation(
                                gate_sb[m - G][:, nsl], gtmp[:], AF.Sigmoid
                            )

                    qgps0_cm = tc.tile_pool(name="qgps0", bufs=1, space="PSUM")
                    qgps0 = qgps0_cm.__enter__()
                    with tc.tile_pool(name="msp", bufs=1, space="PSUM") as msp:
                        ms_ps = [msp.tile([1, MSW], F32, name=f"ms{n}",
                                          uniquify=False) for n in range(MSCH)]
                        # ht[0] leads on the sync queue (the k=0 chain+square
                        # need it first) while wq_0 streams on scalar with a
                        # small head piece; remaining tiles round-robin
                        ht_engs = [nc.sync, nc.gpsimd, nc.scalar]
                        ht_engs[0].dma_start(
                            out=ht[0][:], in_=hst[0:P, :]
                        )
                        wq_0 = load_wq(0, split=True)
                        for k in range(1, KH):
                            ht_engs[k % 3].dma_start(
                                out=ht[k][:], in_=hst[k * P:(k + 1) * P, :]
                            )
                        # wq_1 is requested now, before the rstd Ln/Exp land
                        # on the ACT queue — otherwise its DMA trigger waits
                        # behind them and the m=1 chains stall ~10us
                        wq_1 = load_wq(1)
                        # tiny warmup collective (after the ht loads so it
                        # doesn't head-of-line-block them): absorbs NRT
                        # collective-channel init + cross-core launch skew;
                        # same byte-size as the real half-head AllGathers
                        nc.gpsimd.dma_start(
                            out=warm_in[:], in_=hst[0:P, 0:S // 2]
                        )
                        nc.gpsimd.collective_compute(
                            "AllGather",
                            mybir.AluOpType.bypass,
                            replica_groups=RG,
                            ins=[warm_in[:].opt()],
                            outs=[warm_out[:].opt()],
                        )
                        # k/v weights in dedicated tiles, streamed on the
                        # otherwise-idle gpsimd queue well before the KV
                        # chains need them
                        wk_sb = kvwp.tile([P, KH, P], BF16, name="wk_sb")
                        wv_sb = kvwp.tile([P, KH, P], BF16, name="wv_sb")
                        nc.gpsimd.dma_start(
                            out=wk_sb[:, :, :],
                            in_=wkt[:].rearrange("p (k j) -> p k j", k=KH),
                        )
                        nc.gpsimd.dma_start(
                            out=wv_sb[:, :, :],
                            in_=wvt[:].rearrange("p (k j) -> p k j", k=KH),
                        )
                        # k-outer chains: all NCH column chunks accumulate in
                        # parallel psum banks so each weight k-slice is loaded
                        # into the PE array once per m (4 matmuls per swap)
                        ps_m0 = [qgps0.tile([P, NW], F32, name=f"psq{n}")
                                 for n in range(NCH)]
                        sq_prev = None
                        for k in range(KH):
                            sq_k = []
                            for n in range(MSCH):
                                sqk = sqp.tile([P, MSW], BF16)
                                src = ht[k][:, n * MSW:(n + 1) * MSW]
                                nc.vector.tensor_mul(sqk[:], src, src)
                                sq_k.append(sqk)
                            for n in range(len(ps_m0)):
                                qg_chain_mm(ps_m0[n], wq_0, k, n)
                            if sq_prev is not None:
                                for n in range(MSCH):
                                    nc.tensor.matmul(
                                        ms_ps[n][:],
                                        ones_sb[:],
                                        sq_prev[n][:],
                                        start=(k == 1),
                                        stop=(k == KH - 1 + 1),
                                    )
                            sq_prev = sq_k
                        for n in range(MSCH):
                            nc.tensor.matmul(
                                ms_ps[n][:], ones_sb[:], sq_prev[n][:],
                                start=False, stop=True,
                            )
                        # rstd = exp(-0.5*ln(ms)) on ACT (Rsqrt is blocked in
                        # bass). All Ln chunks run before all Exp chunks so
                        # the ACT table loads twice, not per chunk; the
                        # broadcast is per-chunk so evacs start early.
                        srow = sqp.tile([1, S], F32, bufs=1)
                        lrow = sqp.tile([1, S], F32, bufs=1)
                        for n in range(MSCH):
                            msl = slice(n * MSW, (n + 1) * MSW)
                            nc.scalar.activation(
                                lrow[:, msl],
                                ms_ps[n][:],
                                AF.Ln,
                                bias=eps_t[:],
                                scale=1.0 / HID,
                            )
                        for n in range(MSCH):
                            msl = slice(n * MSW, (n + 1) * MSW)
                            nc.scalar.activation(
                                srow[:, msl], lrow[:, msl], AF.Exp, scale=-0.5
                            )
                            nc.gpsimd.partition_broadcast(
                                rstd_bc[:, msl], srow[:, msl]
                            )
                        for n in range(len(ps_m0)):
                            evac(ps_m0[n], 0, n)
                    qgps0_cm.__exit__(None, None, None)
                    qgps_cm = tc.tile_pool(name="qgps", bufs=2, space="PSUM")
                    qgps = qgps_cm.__enter__()

                    for m in range(1, 2 * G):
                        wq_m = wq_1 if m == 1 else load_wq(m)
                        pss = [qgps.tile([P, NW], F32, name=f"psq_r{n}")
                               for n in range(NCH)]
                        for k in range(KH):
                            for n in range(NCH):
                                qg_chain_mm(pss[n], wq_m, k, n)
                        for n in range(NCH):
                            evac(pss[n], m, n)

                    qgps_cm.__exit__(None, None, None)
                    with tc.tile_pool(name="kvps", bufs=2, space="PSUM") as kvps:
                     for dst_sb, w_sb in ((kt_sb, wk_sb), (vt_sb, wv_sb)):
                        pss = [kvps.tile([P, NW], F32, name=f"ps_kv{n}")
                               for n in range(NCH)]
                        for k in range(KH):
                            for n in range(NCH):
                                nc.tensor.matmul(
                                    pss[n][:],
                                    w_sb[:, k, :],
                                    ht[k][:, n * NW:(n + 1) * NW],
                                    start=(k == 0),
                                    stop=(k == KH - 1),
                                )
                        for n in range(NCH):
                            nc.vector.tensor_mul(
                                dst_sb[:, n * NW:(n + 1) * NW],
                                pss[n][:],
                                rstd_bc[:, n * NW:(n + 1) * NW],
                            )

                # v natural layout via PE transpose of vT tiles
                with tc.tile_pool(name="tpps", bufs=2, space="PSUM") as tpps:
                    for sk in range(NSK):
                        pst = tpps.tile([P, P], BF16)
                        nc.tensor.transpose(
                            pst[:], vt_sb[:, sk * P:(sk + 1) * P], ident_sb[:]
                        )
                        nc.vector.tensor_copy(vnat[sk][:], pst[:])

            # ---- phases 3+4 pools (allocated in the freed hsT zone) ----
            with ExitStack() as ph34:
                wo_bfp = ph34.enter_context(tc.tile_pool(name="wo_bf", bufs=1))
                wo_bf = [wo_bfp.tile([P, HQ], BF16, name=f"wo{kf}", uniquify=False)
                         for kf in range(KH)]
                of_pool = ph34.enter_context(tc.tile_pool(name="of", bufs=1))
                of = [of_pool.tile([P, S], BF16, name=f"of{i}", uniquify=False)
                      for i in range(KH)]
                for kf in range(KH):
                    dma_eng = nc.sync if kf % 2 == 0 else nc.scalar
                    dma_eng.dma_start(
                        out=wo_bf[kf][:], in_=wot[kf * P:(kf + 1) * P, :]
                    )

                # ---- phase 3: attention ----
                with tc.tile_pool(
                    name="pt", bufs=3
                ) as ptp, tc.tile_pool(name="og", bufs=2) as ogp, tc.tile_pool(
                    name="sps", bufs=2, space="PSUM"
                ) as sps, tc.tile_pool(
                    name="ops", bufs=2, space="PSUM"
                ) as ops, tc.tile_pool(name="sums", bufs=2, space="PSUM") as sums:
                    def emit_of_loads(hh):
                        # deferred until the exchange is surely complete, so
                        # these never head-of-line-block the gpsimd queue
                        for half in range(2):
                            hsl = slice(half * (S // 2), (half + 1) * (S // 2))
                            for r in range(4):
                                nc.gpsimd.dma_start(
                                    out=of[hh * 4 + r][:, hsl],
                                    in_=ag_out[hh][half][r * P:(r + 1) * P, :],
                                )

                    for h in range(G):
                        for sqc in range(4):
                            ssl = slice(sqc * SQCH, (sqc + 1) * SQCH)
                            if sqc == 2 and h > 0:
                                emit_of_loads(h - 1)
                            ps_o = ops.tile([P, SQCH], F32)
                            ps_sum = sums.tile([P, SQCH], F32)

                            # pairs of sk tiles share one 2-bank psum + one
                            # exp; p@v of pair skp-1 is emitted after the
                            # scores of pair skp so the PE never sits behind
                            # the ACT exp on the critical path. The pair is
                            # pre-added on DVE so the rowsum matvec streams
                            # once per pair instead of once per sk tile.
                            def emit_pv(skp, pt, pts):
                                for j in range(2):
                                    sk = 2 * skp + j
                                    nc.tensor.matmul(
                                        ps_o[:],
                                        vnat[sk][:],
                                        pt[:, j, :],
                                        start=(sk == 0),
                                        stop=(sk == NSK - 1),
                                    )
                                nc.tensor.matmul(
                                    ps_sum[:],
                                    ones_mat[:],
                                    pts[:],
                                    start=(skp == 0),
                                    stop=(skp == NSK // 2 - 1),
                                )

                            pend = None
                            for skp in range(NSK // 2):
                                ps_s = sps.tile([P, 2, SQCH], F32)
                                for j in range(2):
                                    sk = 2 * skp + j
                                    nc.tensor.matmul(
                                        ps_s[:, j, :],
                                        kt_sb[:, sk * P:(sk + 1) * P],
                                        qt_sb[h][:, ssl],
                                        start=True,
                                        stop=True,
                                    )
                                pt = ptp.tile([P, 2, SQCH], BF16)
                                nc.scalar.activation(
                                    pt[:], ps_s[:], AF.Exp, scale=SCALE
                                )
                                pts = ptp.tile([P, SQCH], BF16, name="pts")
                                nc.vector.tensor_add(
                                    pts[:], pt[:, 0, :], pt[:, 1, :]
                                )
                                if pend is not None:
                                    emit_pv(*pend)
                                pend = (skp, pt, pts)
                            emit_pv(*pend)

                            # normalization: gate already sigmoided in phase
                            # 2, so t1 = ps_o*sig frees the psum bank with no
                            # dependence on the rowsum path; the rowsum
                            # broadcast stays on-chip via partition_broadcast
                            # (no DRAM round trip, no sync-queue coupling)
                            t1 = ogp.tile([P, SQCH], F32)
                            nc.vector.tensor_mul(
                                t1[:], ps_o[:], gate_sb[h][:, ssl]
                            )
                            rden = ogp.tile([P, SQCH], F32)
                            nc.vector.reciprocal(rden[:], ps_sum[:])
                            og = ogp.tile([P, SQCH], BF16)
                            nc.vector.tensor_mul(og[:], t1[:], rden[:])
                            nc.sync.dma_start(
                                out=ag_in[h][sqc // 2][
                                    :, (sqc % 2) * SQCH:(sqc % 2 + 1) * SQCH
                                ],
                                in_=og[:],
                            )
                            if sqc % 2 == 1:
                                half = sqc // 2
                                nc.gpsimd.collective_compute(
                                    "AllGather",
                                    mybir.AluOpType.bypass,
                                    replica_groups=RG,
                                    ins=[ag_in[h][half][:].opt()],
                                    outs=[ag_out[h][half][:].opt()],
                                )
                    emit_of_loads(G - 1)

                # ---- phase 4: O projection (bf16), my HID column quarter ----
                # phase A contracts heads 0..2 (kf 0..11) into SBUF right
                # after head-3 attention, without waiting on the last
                # AllGather; phase B adds head 3 and writes out
                with tc.tile_pool(name="oacc", bufs=1) as oaccp, \
                        tc.tile_pool(name="oev", bufs=3) as oevp:
                    NM = HQ // P
                    OW = min(512, S)  # o-proj chunk (psum: NM banks/set)
                    OCH = S // OW
                    oacc = [
                        oaccp.tile([P, OW], F32, name=f"oacc{n}_{m}")
                        for n in range(OCH) for m in range(NM)
                    ]
                    # phase A runs on 2-matmul psum sets (4 banks total): it
                    # can grab the freed score psum banks and start while the
                    # last head's og tail still holds the ops/sums banks
                    with tc.tile_pool(name="outpsA", bufs=2, space="PSUM") \
                            as outpsA:
                        for n in range(OCH):
                            for mh in range(2):
                                pss = [
                                    outpsA.tile([P, OW], F32, name=f"opsA{mi}")
                                    for mi in range(2)
                                ]
                                for kf in range(12):
                                    for mi in range(2):
                                        m = mh * 2 + mi
                                        nc.tensor.matmul(
                                            pss[mi][:],
                                            wo_bf[kf][:, m * P:(m + 1) * P],
                                            of[kf][:, n * OW:(n + 1) * OW],
                                            start=(kf == 0),
                                            stop=(kf == 11),
                                        )
                                for mi in range(2):
                                    nc.vector.tensor_copy(
                                        oacc[n * NM + mh * 2 + mi][:],
                                        pss[mi][:],
                                    )
                    outps_cm = tc.tile_pool(name="outps", bufs=2, space="PSUM")
                    outps = outps_cm.__enter__()
                    for n in range(OCH):
                        pss = [outps.tile([P, OW], F32, name=f"ops{m}")
                               for m in range(NM)]
                        for kf in range(12, KH):
                            for m in range(NM):
                                nc.tensor.matmul(
                                    pss[m][:],
                                    wo_bf[kf][:, m * P:(m + 1) * P],
                                    of[kf][:, n * OW:(n + 1) * OW],
                                    start=(kf == 12),
                                    stop=(kf == KH - 1),
                                )
                        for m in range(NM):
                            oev = oevp.tile([P, OW], F32)
                            nc.vector.tensor_add(
                                oev[:], pss[m][:], oacc[n * NM + m][:]
                            )
                            dma_eng = nc.sync if (n + m) % 2 == 0 else nc.scalar
                            dma_eng.dma_start(
                                out=out[m * P:(m + 1) * P, n * OW:(n + 1) * OW],
                                in_=oev[:],
                            )
                    outps_cm.__exit__(None, None, None)

    nc.compile()
    return nc


def make_in_maps(hidden_states, Wq, Wk, Wv, Wo, norm_w, S=S_FULL):
    """Host-side sharding/layout prep. Core c -> (batch c//4, rank c%4)."""
    w1p = (1.0 + norm_w).astype(np.float32)
    WqT = np.ascontiguousarray((Wq * w1p[None, :]).T)  # [HID, 2*NH*HD]
    WkT = np.ascontiguousarray((Wk * w1p[None, :]).T)  # [HID, NKV*HD]
    WvT = np.ascontiguousarray((Wv * w1p[None, :]).T)
    WoT = np.ascontiguousarray(Wo.T)  # [NH*HD, HID]
    # permute feat blocks to match AG stacking: pos h*4+r holds head 4r+h
    perm = [4 * (p % 4) + p // 4 for p in range(NH)]
    WoTp = np.ascontiguousarray(
        WoT.reshape(NH, HD, HID)[perm].reshape(NH * HD, HID)
    )
    ident = np.eye(P, dtype=np.float32)

    def tile_w(wt):
        # [HID, C] -> per 128-col block m: [P, KH*P] with wq_m[p, k*P+j] =
        # wt[k*P+p, m*P+j]
        C = wt.shape[1]
        blocks = []
        for m in range(C // P):
            blk = wt[:, m * P:(m + 1) * P].reshape(KH, P, P)
            blocks.append(blk.transpose(1, 0, 2).reshape(P, KH * P))
        return np.ascontiguousarray(np.stack(blocks))

    import ml_dtypes

    bf = ml_dtypes.bfloat16
    in_maps = []
    for c in range(N_CORES):
        b, r = c // 4, c % 4
        qcols = np.r_[r * 512:(r + 1) * 512, NH * HD + r * 512:NH * HD + (r + 1) * 512]
        in_maps.append(
            {
                "hst": np.ascontiguousarray(hidden_states[b, :S].T.astype(bf)),
                "wqt": tile_w(WqT[:, qcols]).astype(bf),
                "wkt": tile_w(WkT[:, r * HD:(r + 1) * HD])[0].astype(bf),
                "wvt": tile_w(WvT[:, r * HD:(r + 1) * HD])[0].astype(bf),
                "wot": np.ascontiguousarray(
                    WoTp[:, r * HQ:(r + 1) * HQ].astype(bf)
                ),
                "identp": ident.astype(bf),
            }
        )
    return in_maps


def gather_out(results, S=S_FULL):
    out = np.empty((B, S, HID), np.float32)
    for c in range(N_CORES):
        b, r = c // 4, c % 4
        out[b, :, r * HQ:(r + 1) * HQ] = results[c]["out"].T
    return out


_NC_CACHE = {}


def kernel(**inputs) -> np.ndarray:
    from concourse.bass_utils import run_bass_kernel_spmd

    hidden_states = np.asarray(inputs["hidden_states"], dtype=np.float32)
    Wq = np.asarray(inputs["Wq"], dtype=np.float32)
    Wk = np.asarray(inputs["Wk"], dtype=np.float32)
    Wv = np.asarray(inputs["Wv"], dtype=np.float32)
    Wo = np.asarray(inputs["Wo"], dtype=np.float32)
    norm_w = np.asarray(inputs["norm_w"], dtype=np.float32)

    if "nc" not in _NC_CACHE:
        _NC_CACHE["nc"] = build()
    nc = _NC_CACHE["nc"]

    in_maps = make_in_maps(hidden_states, Wq, Wk, Wv, Wo, norm_w)
    res = run_bass_kernel_spmd(nc, in_maps, list(range(N_CORES)))
    return gather_out(res.results)



# revision 68
# speedup vs baseline: 1.1268x; 1.0119x over previous
"""Trainium2 Bass kernel for nn_Attention_3556232921308.

GQA attention layer: RMSNorm -> {Q+gate, K, V} proj -> softmax attention
(no mask, no rope) -> sigmoid output gate -> O proj.
B=2, S=2048, HID=2048, NH=16, NKV=4, HD=128.

Sharding (8 cores): DP over batch (2 groups of 4 cores) x TP over KV heads
(4 ranks per group; each rank owns 1 KV head = 4 Q/gate heads). The output
projection contracts over all heads, so gated attention outputs (bf16) are
exchanged with per-head AllGathers; each rank then computes the O-projection
for all tokens but only its quarter of the HID output columns (the Wo slice
is host-provided per rank, keeping the SPMD graph rank-independent).

Host-side prep: transposes (activations/weights enter the PE contracted
over the partition dim), folding the RMSNorm (1+w) scale into the
projection weights, pre-tiling weights into [P, KH*P] blocks so each loads
with one linear DMA, and casting hidden states + weights to bf16 (all
matmuls run bf16; measured end-to-end rel err ~5.6e-3 vs the 2e-2 gate).

Compute layout notes:
 - hidden states live as hsT [HID, S]; mean-square is a ones-matvec on PE
   (squares on DVE), rstd = exp(-0.5*ln(ms)) on ACT (Rsqrt is blocked in
   bass), applied to the *outputs* of the raw projections.
 - projections run k-outer: all 4 column-chunk psum chains in parallel so
   each weight k-slice is loaded into the PE once per output block.
 - q/gate stay SBUF-resident; the gate sigmoid is applied at evacuation on
   ACT (one table load), so the attention hot loop is exp-only on ACT.
 - q/k are produced directly in [HD, S] (per head) layout, so scores^T
   [Sk, Sq] needs no transposes; v is PE-transposed per 128-tile.
 - exp(scores) runs on ACT straight out of PSUM with the 1/sqrt(HD) scale
   folded in; no max-subtraction (|scores| < 8 here; fp32 exp is safe).
   Softmax denominators: exp pairs pre-added on DVE, then a matvec against
   an all-ones [128,128] stationary writes the rowsum broadcast across all
   psum partitions (matmul cost scales with moving columns only), so
   normalization is recip+mul on DVE with no broadcast plumbing.
 - every head's AllGather is split into column halves so the exchanges
   (~20us each) hide under attention compute; the AG-dependent of-loads
   are deferred and isolated on the gpsimd queue so they never head-of-
   line-block a compute-feeding queue.
"""
import math
from contextlib import ExitStack

import numpy as np

B, S_FULL, HID = 2, 2048, 2048
NH, NKV, HD = 16, 4, 128
G = NH // NKV  # 4 q heads per kv head = heads per rank
EPS = 1e-6
N_CORES = 8
P = 128
KH = HID // P  # 16 contraction tiles
HQ = HID // 4  # per-rank output column quarter (512)


def build(S=S_FULL):
    import concourse.bass as bass  # noqa: F401
    import concourse.tile as tile
    from concourse import bacc, mybir

    F32R = mybir.dt.float32r
    F32 = mybir.dt.float32
    BF16 = mybir.dt.bfloat16
    AF = mybir.ActivationFunctionType

    SQCH = S // 4  # attention sq chunk
    NW = min(512, S)  # projection free-dim chunk (psum bank = 512 fp32)
    NCH = S // NW
    MSW = NW  # mean-square matvec chunk
    MSCH = NCH
    NSK = S // P  # score key tiles
    HPR = G * HD  # feats per rank for q/gate (512)
    SCALE = 1.0 / math.sqrt(HD)
    RG = [[0, 1, 2, 3], [4, 5, 6, 7]]

    nc = bacc.Bacc("TRN2", target_bir_lowering=False, debug=False, num_devices=N_CORES)

    hst = nc.declare_dram_parameter("hst", [HID, S], BF16, isOutput=False)
    # weights ship pre-tiled as [P, KH*P] blocks (one linear DMA each)
    wqt = nc.declare_dram_parameter("wqt", [2 * G, P, KH * P], BF16, isOutput=False)
    wkt = nc.declare_dram_parameter("wkt", [P, KH * P], BF16, isOutput=False)
    wvt = nc.declare_dram_parameter("wvt", [P, KH * P], BF16, isOutput=False)
    wot = nc.declare_dram_parameter("wot", [NH * HD, HQ], BF16, isOutput=False)
    identp = nc.declare_dram_parameter("identp", [P, P], BF16, isOutput=False)
    out = nc.declare_dram_parameter("out", [HQ, S], F32, isOutput=True)

    with tile.TileContext(nc) as tc, ExitStack() as ctx:
        dram = ctx.enter_context(tc.tile_pool(name="dram", bufs=1, space="DRAM"))
        # every head's AllGather is split into two column-halves: the first
        # half launches mid-head and its ~20us exchange hides under the rest
        # of the head's attention, so neither the next head nor the final
        # O-projection ever waits on a full 2MB end-of-head exchange
        ag_in = [
            [
                dram.tile([P, S // 2], BF16, name=f"ag_in{h}_{i}", uniquify=False)
                for i in range(2)
            ]
            for h in range(G)
        ]
        ag_out = [
            [
                dram.tile(
                    [4 * P, S // 2], BF16, name=f"ag_out{h}_{i}", uniquify=False
                )
                for i in range(2)
            ]
            for h in range(G)
        ]

        warm_in = dram.tile([P, S // 2], BF16)
        warm_out = dram.tile([4 * P, S // 2], BF16)

        consts = ctx.enter_context(tc.tile_pool(name="consts", bufs=1))
        ones_sb = consts.tile([P, 1], BF16)
        nc.vector.memset(ones_sb[:], 1.0)
        # all-ones stationary: the softmax rowsum matvec writes its result
        # broadcast across all 128 psum partitions (same cost: matmul time
        # scales with moving columns only), so no copy/broadcast plumbing
        ones_mat = consts.tile([P, P], BF16)
        nc.vector.memset(ones_mat[:], 1.0)
        ident_sb = consts.tile([P, P], BF16)
        nc.gpsimd.dma_start(out=ident_sb[:], in_=identp[:])
        rstd_bc = consts.tile([P, S], F32)
        eps_t = consts.tile([1, 1], F32)
        nc.vector.memset(eps_t[:], EPS)

        # PE pre-warm: a burst of 1-column dummy matmuls while the first
        # weight/activation DMAs stream in, so the HAM clock gate reaches
        # 8/8 before the first real chain issues (saves the 1.2GHz ramp)
        with tc.tile_pool(name="pwm", bufs=1, space="PSUM") as pwmp:
            pwt = pwmp.tile([1, 1], F32)
            for _ in range(40):
                nc.tensor.matmul(
                    pwt[:], ones_sb[:, :1], ones_sb[:, :1],
                    start=True, stop=True,
                )

        with ExitStack() as ph123:
            kv_pool = ph123.enter_context(tc.tile_pool(name="kv", bufs=1))
            kt_sb = kv_pool.tile([P, S], BF16)
            vnat = [kv_pool.tile([P, P], BF16, name=f"vnat{i}", uniquify=False)
                    for i in range(NSK)]
            # q/gate stay SBUF-resident (evac writes land here directly;
            # no DRAM round trip, no attention-phase reload DMAs)
            qt_sb = [kv_pool.tile([P, S], BF16, name=f"qt{m}", uniquify=False)
                     for m in range(G)]
            gate_sb = [kv_pool.tile([P, S], BF16, name=f"gt{m}", uniquify=False)
                       for m in range(G)]

            # ---- phases 1+2: norm stats + projections (hsT resident) ----
            with ExitStack() as ph:
                ht_pool = ph.enter_context(tc.tile_pool(name="ht", bufs=1))
                ht = [ht_pool.tile([P, S], BF16, name=f"ht{k}", uniquify=False)
                      for k in range(KH)]
                vt_sb = ht_pool.tile([P, S], BF16)

                # norm stats + projections. The mean-square matvecs are
                # software-pipelined one k behind their squares (half on
                # ACT, half on DVE), with the m=0 q-projection psum chains
                # interleaved so the PE never idles waiting on squares.
                with tc.tile_pool(name="sqp", bufs=4) as sqp, tc.tile_pool(
                    name="wq", bufs=2
                ) as wqp, tc.tile_pool(name="kvw", bufs=1) as kvwp:

                    def load_wq(m, split=False):
                        wq_m = wqp.tile([P, KH, P], BF16, name="wq_m", tag="wq_m")
                        dma_eng = nc.sync if m % 2 == 0 else nc.scalar
                        src = wqt[m].rearrange("p (k j) -> p k j", k=KH)
                        if split:
                            # small head piece first so the k=0 chain step
                            # can fire while the bulk is still streaming
                            nc.scalar.dma_start(
                                out=wq_m[:, 0:2, :], in_=src[:, 0:2, :]
                            )
                            nc.scalar.dma_start(
                                out=wq_m[:, 2:, :], in_=src[:, 2:, :]
                            )
                        else:
                            dma_eng.dma_start(out=wq_m[:, :, :], in_=src)
                        return wq_m

                    def qg_chain_mm(ps, wq_m, k, n):
                        nc.tensor.matmul(
                            ps[:],
                            wq_m[:, k, :],
                            ht[k][:, n * NW:(n + 1) * NW],
                            start=(k == 0),
                            stop=(k == KH - 1),
                        )

                    def evac(ps, m, n):
                        nsl = slice(n * NW, (n + 1) * NW)
                        if m < G:
                            nc.vector.tensor_mul(
                                qt_sb[m][:, nsl], ps[:], rstd_bc[:, nsl]
                            )
                        else:
                            # gate heads: apply the sigmoid here on ACT (one
                            # table load for all of phase 2) so the attention
                            # hot loop never touches the gate nonlinearity
                            gtmp = sqp.tile([P, NW], F32, name="gtmp")
                            nc.vector.tensor_mul(
                                gtmp[:], ps[:], rstd_bc[:, nsl]
                            )
                            nc.scalar.activation(
                                gate_sb[m - G][:, nsl], gtmp[:], AF.Sigmoid
                            )

                    qgps0_cm = tc.tile_pool(name="qgps0", bufs=1, space="PSUM")
                    qgps0 = qgps0_cm.__enter__()
                    with tc.tile_pool(name="msp", bufs=1, space="PSUM") as msp:
                        ms_ps = [msp.tile([1, MSW], F32, name=f"ms{n}",
                                          uniquify=False) for n in range(MSCH)]
                        # ht[0] leads on the sync queue (the k=0 chain+square
                        # need it first) while wq_0 streams on scalar with a
                        # small head piece; remaining tiles round-robin
                        ht_engs = [nc.sync, nc.gpsimd, nc.scalar]
                        ht_engs[0].dma_start(
                            out=ht[0][:], in_=hst[0:P, :]
                        )
                        wq_0 = load_wq(0, split=True)
                        for k in range(1, KH):
                            ht_engs[k % 3].dma_start(
                                out=ht[k][:], in_=hst[k * P:(k + 1) * P, :]
                            )
                        # wq_1 is requested now, before the rstd Ln/Exp land
                        # on the ACT queue — otherwise its DMA trigger waits
                        # behind them and the m=1 chains stall ~10us
                        wq_1 = load_wq(1)
                        # tiny warmup collective (after the ht loads so it
                        # doesn't head-of-line-block them): absorbs NRT
                        # collective-channel init + cross-core launch skew;
                        # same byte-size as the real half-head AllGathers
                        nc.gpsimd.dma_start(
                            out=warm_in[:], in_=hst[0:P, 0:S // 2]
                        )
                        nc.gpsimd.collective_compute(
                            "AllGather",
                            mybir.AluOpType.bypass,
                            replica_groups=RG,
                            ins=[warm_in[:].opt()],
                            outs=[warm_out[:].opt()],
                        )
                        # k/v weights in dedicated tiles, streamed on the
                        # otherwise-idle gpsimd queue well before the KV
                        # chains need them
                        wk_sb = kvwp.tile([P, KH, P], BF16, name="wk_sb")
                        wv_sb = kvwp.tile([P, KH, P], BF16, name="wv_sb")
                        nc.gpsimd.dma_start(
                            out=wk_sb[:, :, :],
                            in_=wkt[:].rearrange("p (k j) -> p k j", k=KH),
                        )
                        nc.gpsimd.dma_start(
                            out=wv_sb[:, :, :],
                            in_=wvt[:].rearrange("p (k j) -> p k j", k=KH),
                        )
                        # k-outer chains: all NCH column chunks accumulate in
                        # parallel psum banks so each weight k-slice is loaded
                        # into the PE array once per m (4 matmuls per swap)
                        ps_m0 = [qgps0.tile([P, NW], F32, name=f"psq{n}")
                                 for n in range(NCH)]
                        sq_prev = None
                        for k in range(KH):
                            sq_k = []
                            for n in range(MSCH):
                                sqk = sqp.tile([P, MSW], BF16)
                                src = ht[k][:, n * MSW:(n + 1) * MSW]
                                nc.vector.tensor_mul(sqk[:], src, src)
                                sq_k.append(sqk)
                            for n in range(len(ps_m0)):
                                qg_chain_mm(ps_m0[n], wq_0, k, n)
                            if sq_prev is not None:
                                for n in range(MSCH):
                                    nc.tensor.matmul(
                                        ms_ps[n][:],
                                        ones_sb[:],
                                        sq_prev[n][:],
                                        start=(k == 1),
                                        stop=(k == KH - 1 + 1),
                                    )
                            sq_prev = sq_k
                        for n in range(MSCH):
                            nc.tensor.matmul(
                                ms_ps[n][:], ones_sb[:], sq_prev[n][:],
                                start=False, stop=True,
                            )
                        # rstd = exp(-0.5*ln(ms)) on ACT (Rsqrt is blocked in
                        # bass). All Ln chunks run before all Exp chunks so
                        # the ACT table loads twice, not per chunk; the
                        # broadcast is per-chunk so evacs start early.
                        srow = sqp.tile([1, S], F32, bufs=1)
                        lrow = sqp.tile([1, S], F32, bufs=1)
                        for n in range(MSCH):
                            msl = slice(n * MSW, (n + 1) * MSW)
                            nc.scalar.activation(
                                lrow[:, msl],
                                ms_ps[n][:],
                                AF.Ln,
                                bias=eps_t[:],
                                scale=1.0 / HID,
                            )
                        for n in range(MSCH):
                            msl = slice(n * MSW, (n + 1) * MSW)
                            nc.scalar.activation(
                                srow[:, msl], lrow[:, msl], AF.Exp, scale=-0.5
                            )
                            nc.gpsimd.partition_broadcast(
                                rstd_bc[:, msl], srow[:, msl]
                            )
                        for n in range(len(ps_m0)):
                            evac(ps_m0[n], 0, n)
                    qgps0_cm.__exit__(None, None, None)
                    qgps_cm = tc.tile_pool(name="qgps", bufs=2, space="PSUM")
                    qgps = qgps_cm.__enter__()

                    for m in range(1, 2 * G):
                        wq_m = wq_1 if m == 1 else load_wq(m)
                        pss = [qgps.tile([P, NW], F32, name=f"psq_r{n}")
                               for n in range(NCH)]
                        for k in range(KH):
                            for n in range(NCH):
                                qg_chain_mm(pss[n], wq_m, k, n)
                        for n in range(NCH):
                            evac(pss[n], m, n)

                    qgps_cm.__exit__(None, None, None)
                    with tc.tile_pool(name="kvps", bufs=2, space="PSUM") as kvps:
                     for dst_sb, w_sb in ((kt_sb, wk_sb), (vt_sb, wv_sb)):
                        pss = [kvps.tile([P, NW], F32, name=f"ps_kv{n}")
                               for n in range(NCH)]
                        for k in range(KH):
                            for n in range(NCH):
                                nc.tensor.matmul(
                                    pss[n][:],
                                    w_sb[:, k, :],
                                    ht[k][:, n * NW:(n + 1) * NW],
                                    start=(k == 0),
                                    stop=(k == KH - 1),
                                )
                        for n in range(NCH):
                            nc.vector.tensor_mul(
                                dst_sb[:, n * NW:(n + 1) * NW],
                                pss[n][:],
                                rstd_bc[:, n * NW:(n + 1) * NW],
                            )

                # v natural layout via PE transpose of vT tiles
                with tc.tile_pool(name="tpps", bufs=2, space="PSUM") as tpps:
                    for sk in range(NSK):
                        pst = tpps.tile([P, P], BF16)
                        nc.tensor.transpose(
                            pst[:], vt_sb[:, sk * P:(sk + 1) * P], ident_sb[:]
                        )
                        nc.vector.tensor_copy(vnat[sk][:], pst[:])

            # ---- phases 3+4 pools (allocated in the freed hsT zone) ----
            with ExitStack() as ph34:
                wo_bfp = ph34.enter_context(tc.tile_pool(name="wo_bf", bufs=1))
                wo_bf = [wo_bfp.tile([P, HQ], BF16, name=f"wo{kf}", uniquify=False)
                         for kf in range(KH)]
                of_pool = ph34.enter_context(tc.tile_pool(name="of", bufs=1))
                of = [of_pool.tile([P, S], BF16, name=f"of{i}", uniquify=False)
                      for i in range(KH)]
                for kf in range(KH):
                    dma_eng = nc.sync if kf % 2 == 0 else nc.scalar
                    dma_eng.dma_start(
                        out=wo_bf[kf][:], in_=wot[kf * P:(kf + 1) * P, :]
                    )

                # ---- phase 3: attention ----
                with tc.tile_pool(
                    name="pt", bufs=3
                ) as ptp, tc.tile_pool(name="og", bufs=2) as ogp, tc.tile_pool(
                    name="sps", bufs=2, space="PSUM"
                ) as sps, tc.tile_pool(
                    name="ops", bufs=2, space="PSUM"
                ) as ops, tc.tile_pool(name="sums", bufs=2, space="PSUM") as sums:
                    def emit_of_loads(hh):
                        # deferred until the exchange is surely complete, so
                        # these never head-of-line-block the gpsimd queue
                        for half in range(2):
                            hsl = slice(half * (S // 2), (half + 1) * (S // 2))
                            for r in range(4):
                                nc.gpsimd.dma_start(
                                    out=of[hh * 4 + r][:, hsl],
                                    in_=ag_out[hh][half][r * P:(r + 1) * P, :],
                                )

                    # pairs of sk tiles share one 2-bank psum + one exp; the
                    # pipeline carries ACROSS sqc boundaries: the next sqc's
                    # first scores issue before the previous sqc's last p@v,
                    # so the PE never drains waiting on the tail exp. Each
                    # sqc's normalization tail is emitted from inside the
                    # next sqc's loop, fully overlapped with its scores.
                    def emit_pv(ent):
                        eh, esqc, skp, pt, pts, ps_o, ps_sum = ent
                        for j in range(2):
                            sk = 2 * skp + j
                            nc.tensor.matmul(
                                ps_o[:],
                                vnat[sk][:],
                                pt[:, j, :],
                                start=(sk == 0),
                                stop=(sk == NSK - 1),
                            )
                        nc.tensor.matmul(
                            ps_sum[:],
                            ones_mat[:],
                            pts[:],
                            start=(skp == 0),
                            stop=(skp == NSK // 2 - 1),
                        )

                    def emit_tail(eh, esqc, ps_o, ps_sum):
                        # gate already sigmoided in phase 2: t1 frees the
                        # psum bank with no dependence on the rowsum path;
                        # ps_sum arrives broadcast across partitions from
                        # the all-ones matvec, so normalization is just
                        # recip+mul on DVE
                        esl = slice(esqc * SQCH, (esqc + 1) * SQCH)
                        t1 = ogp.tile([P, SQCH], F32)
                        nc.vector.tensor_mul(
                            t1[:], ps_o[:], gate_sb[eh][:, esl]
                        )
                        rden = ogp.tile([P, SQCH], F32)
                        nc.vector.reciprocal(rden[:], ps_sum[:])
                        og = ogp.tile([P, SQCH], BF16)
                        nc.vector.tensor_mul(og[:], t1[:], rden[:])
                        nc.sync.dma_start(
                            out=ag_in[eh][esqc // 2][
                                :, (esqc % 2) * SQCH:(esqc % 2 + 1) * SQCH
                            ],
                            in_=og[:],
                        )
                        if esqc % 2 == 1:
                            half = esqc // 2
                            nc.gpsimd.collective_compute(
                                "AllGather",
                                mybir.AluOpType.bypass,
                                replica_groups=RG,
                                ins=[ag_in[eh][half][:].opt()],
                                outs=[ag_out[eh][half][:].opt()],
                            )

                    def flush(pend):
                        emit_pv(pend)
                        if pend[2] == NSK // 2 - 1:
                            emit_tail(pend[0], pend[1], pend[5], pend[6])

                    pend = None
                    for h in range(G):
                        for sqc in range(4):
                            ssl = slice(sqc * SQCH, (sqc + 1) * SQCH)
                            if sqc == 2 and h > 0:
                                emit_of_loads(h - 1)
                            ps_o = ops.tile([P, SQCH], F32)
                            ps_sum = sums.tile([P, SQCH], F32)
                            for skp in range(NSK // 2):
                                ps_s = sps.tile([P, 2, SQCH], F32)
                                for j in range(2):
                                    sk = 2 * skp + j
                                    nc.tensor.matmul(
                                        ps_s[:, j, :],
                                        kt_sb[:, sk * P:(sk + 1) * P],
                                        qt_sb[h][:, ssl],
                                        start=True,
                                        stop=True,
                                    )
                                pt = ptp.tile([P, 2, SQCH], BF16)
                                nc.scalar.activation(
                                    pt[:], ps_s[:], AF.Exp, scale=SCALE
                                )
                                pts = ptp.tile([P, SQCH], BF16, name="pts")
                                nc.vector.tensor_add(
                                    pts[:], pt[:, 0, :], pt[:, 1, :]
                                )
                                if pend is not None:
                                    flush(pend)
                                pend = (h, sqc, skp, pt, pts, ps_o, ps_sum)
                    flush(pend)
                    emit_of_loads(G - 1)

                # ---- phase 4: O projection (bf16), my HID column quarter ----
                # phase A contracts heads 0..2 (kf 0..11) into SBUF right
                # after head-3 attention, without waiting on the last
                # AllGather; phase B adds head 3 and writes out
                with tc.tile_pool(name="oacc", bufs=1) as oaccp, \
                        tc.tile_pool(name="oev", bufs=3) as oevp:
                    NM = HQ // P
                    OW = min(512, S)  # o-proj chunk (psum: NM banks/set)
                    OCH = S // OW
                    oacc = [
                        oaccp.tile([P, OW], F32, name=f"oacc{n}_{m}")
                        for n in range(OCH) for m in range(NM)
                    ]
                    # phase A runs on 2-matmul psum sets (4 banks total): it
                    # can grab the freed score psum banks and start while the
                    # last head's og tail still holds the ops/sums banks
                    with tc.tile_pool(name="outpsA", bufs=2, space="PSUM") \
                            as outpsA:
                        for n in range(OCH):
                            for mh in range(2):
                                pss = [
                                    outpsA.tile([P, OW], F32, name=f"opsA{mi}")
                                    for mi in range(2)
                                ]
                                for kf in range(12):
                                    for mi in range(2):
                                        m = mh * 2 + mi
                                        nc.tensor.matmul(
                                            pss[mi][:],
                                            wo_bf[kf][:, m * P:(m + 1) * P],
                                            of[kf][:, n * OW:(n + 1) * OW],
                                            start=(kf == 0),
                                            stop=(kf == 11),
                                        )
                                for mi in range(2):
                                    nc.vector.tensor_copy(
                                        oacc[n * NM + mh * 2 + mi][:],
                                        pss[mi][:],
                                    )
                    outps_cm = tc.tile_pool(name="outps", bufs=2, space="PSUM")
                    outps = outps_cm.__enter__()
                    for n in range(OCH):
                        pss = [outps.tile([P, OW], F32, name=f"ops{m}")
                               for m in range(NM)]
                        for kf in range(12, KH):
                            for m in range(NM):
                                nc.tensor.matmul(
                                    pss[m][:],
                                    wo_bf[kf][:, m * P:(m + 1) * P],
                                    of[kf][:, n * OW:(n + 1) * OW],
                                    start=(kf == 12),
                                    stop=(kf == KH - 1),
                                )
                        for m in range(NM):
                            oev = oevp.tile([P, OW], F32)
                            nc.vector.tensor_add(
                                oev[:], pss[m][:], oacc[n * NM + m][:]
                            )
                            dma_eng = nc.sync if (n + m) % 2 == 0 else nc.scalar
                            dma_eng.dma_start(
                                out=out[m * P:(m + 1) * P, n * OW:(n + 1) * OW],
                                in_=oev[:],
                            )
                    outps_cm.__exit__(None, None, None)

    nc.compile()
    return nc


def make_in_maps(hidden_states, Wq, Wk, Wv, Wo, norm_w, S=S_FULL):
    """Host-side sharding/layout prep. Core c -> (batch c//4, rank c%4)."""
    w1p = (1.0 + norm_w).astype(np.float32)
    WqT = np.ascontiguousarray((Wq * w1p[None, :]).T)  # [HID, 2*NH*HD]
    WkT = np.ascontiguousarray((Wk * w1p[None, :]).T)  # [HID, NKV*HD]
    WvT = np.ascontiguousarray((Wv * w1p[None, :]).T)
    WoT = np.ascontiguousarray(Wo.T)  # [NH*HD, HID]
    # permute feat blocks to match AG stacking: pos h*4+r holds head 4r+h
    perm = [4 * (p % 4) + p // 4 for p in range(NH)]
    WoTp = np.ascontiguousarray(
        WoT.reshape(NH, HD, HID)[perm].reshape(NH * HD, HID)
    )
    ident = np.eye(P, dtype=np.float32)

    def tile_w(wt):
        # [HID, C] -> per 128-col block m: [P, KH*P] with wq_m[p, k*P+j] =
        # wt[k*P+p, m*P+j]
        C = wt.shape[1]
        blocks = []
        for m in range(C // P):
            blk = wt[:, m * P:(m + 1) * P].reshape(KH, P, P)
            blocks.append(blk.transpose(1, 0, 2).reshape(P, KH * P))
        return np.ascontiguousarray(np.stack(blocks))

    import ml_dtypes

    bf = ml_dtypes.bfloat16
    in_maps = []
    for c in range(N_CORES):
        b, r = c // 4, c % 4
        qcols = np.r_[r * 512:(r + 1) * 512, NH * HD + r * 512:NH * HD + (r + 1) * 512]
        in_maps.append(
            {
                "hst": np.ascontiguousarray(hidden_states[b, :S].T.astype(bf)),
                "wqt": tile_w(WqT[:, qcols]).astype(bf),
                "wkt": tile_w(WkT[:, r * HD:(r + 1) * HD])[0].astype(bf),
                "wvt": tile_w(WvT[:, r * HD:(r + 1) * HD])[0].astype(bf),
                "wot": np.ascontiguousarray(
                    WoTp[:, r * HQ:(r + 1) * HQ].astype(bf)
                ),
                "identp": ident.astype(bf),
            }
        )
    return in_maps


def gather_out(results, S=S_FULL):
    out = np.empty((B, S, HID), np.float32)
    for c in range(N_CORES):
        b, r = c // 4, c % 4
        out[b, :, r * HQ:(r + 1) * HQ] = results[c]["out"].T
    return out


_NC_CACHE = {}


def kernel(**inputs) -> np.ndarray:
    from concourse.bass_utils import run_bass_kernel_spmd

    hidden_states = np.asarray(inputs["hidden_states"], dtype=np.float32)
    Wq = np.asarray(inputs["Wq"], dtype=np.float32)
    Wk = np.asarray(inputs["Wk"], dtype=np.float32)
    Wv = np.asarray(inputs["Wv"], dtype=np.float32)
    Wo = np.asarray(inputs["Wo"], dtype=np.float32)
    norm_w = np.asarray(inputs["norm_w"], dtype=np.float32)

    if "nc" not in _NC_CACHE:
        _NC_CACHE["nc"] = build()
    nc = _NC_CACHE["nc"]

    in_maps = make_in_maps(hidden_states, Wq, Wk, Wv, Wo, norm_w)
    res = run_bass_kernel_spmd(nc, in_maps, list(range(N_CORES)))
    return gather_out(res.results)

